# revision 51
# baseline (speedup 1.0000x reference)
"""GCN message-passing + global-sum-pool + dense sigmoid head on 8 NeuronCores.

Math: the reference computes
    h = x @ W1; msg = h[src] * ew; agg = segment_sum(msg, dst) + b1
    pooled = sum(agg, axis=0); out = sigmoid(pooled @ Wd + bd)
Summing a segment_sum over all segments is the sum over all edges, so dst
drops out and by linearity the network collapses exactly to
    logit = sum_e ew[e] * y[src[e]] + N*(b1 @ Wd) + bd,   y = x @ (W1 @ Wd)
    out   = sigmoid(logit)

Distribution: edges are sharded by src range (core c owns nodes
[6250c, 6250(c+1)) and every edge whose src falls there); the tiny dense
head is replicated.  Per core the edge sum is a w-weighted histogram over
the 6250 local nodes contracted with y.

The histogram uses a narrow 16-wide one-hot: local node n = g*16 + b
(g = n>>4 in [0,391), b = n&15).  The host sorts each core's edges by g
and pads every group g to a uniform slot count K_g = max over cores of
that group's edge count (so the slot->tile->psum-column map is identical
on all 8 cores -- required because the NEFF is SPMD).  Each 128-slot tile
then covers 1-2 consecutive groups; its matmul
    s[b, g0:g0+span] += v01_t^T @ wsplit_t
has lhsT = v01_t [128 slots, 16] (DVE is_equal one-hot of b, fp16 2x
mode, 16x cheaper than a 128-wide one-hot + w-scale) and rhs = wsplit_t
[128, span] (host-routed w into the group column, zero elsewhere).
Tiles round-robin over the four 32-wide PE column groups (tile_position
via out base partition 32j) so the ~853 small matmuls run 4-way
concurrent, each into its own PSUM bank.  y[n] = (x @ u)[n] (u = W1@Wd,
computed on device) is produced as y128[b7, a] (n = a*128 + b7) by 196
M=32 matmuls spread over the same four column groups and interleaved
into the histogram stream, then remapped through DRAM into y2[32j+b, g]
replicated per column group; logit partial = sum_j sum s_j * y2.  The 8
scalar partials are AllGathered and every core computes the sigmoid head
redundantly; the host takes core 0's output.
"""

import sys

import numpy as np

sys.path.insert(0, "/opt/trn_rl_repo")

from concourse import bacc, bass, mybir, tile  # noqa: E402
from concourse.bass_utils import run_bass_kernel_spmd  # noqa: E402

N_NODES = 50000
N_EDGES = 800000
N_FEAT = 64
NC = 8
P = 128

NSH = N_NODES // NC            # 6250 nodes per core
GSH = 16                       # nodes per histogram group (one-hot width)
AOH = 49                       # y128 column blocks (6272 = 49*128 padded nodes)
XTW = AOH * P                  # x shard padded to 6272 node columns
# group id g = q*49 + a for local node n = a*128 + q*16 + b (a<49, q<8,
# b<16): this q-major numbering makes the histogram's psum-column order
# match a 3-dim contiguous DMA read of the y table (y_dr[q, b, a]).
NG = 8 * AOH                   # 392 group columns (the tail ones empty)
EP = 2                         # edges per slot (same node, shared one-hot)
TB = 107                       # tiles per batched DVE one-hot build

F32 = mybir.dt.float32
F16 = mybir.dt.float16
I16 = mybir.dt.int16

_cache: dict = {}


class _Layout:
    """Static (SPMD-shared) slot/tile/column map derived from per-group
    PAIR-slot maxima over the 8 cores.  A slot holds up to EP edges of one
    node (same one-hot row); psum flat column c = EP*g + r."""

    def __init__(self, kg: np.ndarray):
        kg = kg.astype(np.int64)
        assert kg.shape == (NG,)
        self.kg = kg
        self.goff = np.concatenate([[0], np.cumsum(kg)])
        self.nslot = int(self.goff[-1])
        self.nt2 = max(1, (self.nslot + P - 1) // P)
        self.nb = (self.nt2 + TB - 1) // TB
        self.nt2p = self.nb * TB
        t = np.arange(self.nt2)
        g_lo = np.searchsorted(self.goff, t * P, side="right") - 1
        last = np.minimum((t + 1) * P, self.nslot) - 1
        g_hi = np.searchsorted(self.goff, last, side="right") - 1
        g_lo = np.minimum(np.maximum(g_lo, 0), NG - 1)
        g_hi = np.minimum(np.maximum(g_hi, g_lo), NG - 1)
        self.g_lo = g_lo
        self.span = (g_hi - g_lo + 1).astype(np.int64)
        self.woff = np.concatenate([[0], np.cumsum(self.span)])
        self.wtot = int(self.woff[-1])
        self.key = hash(kg.tobytes())


def _build(reps, lay: _Layout, acc=False, debug_outs=False, sj=4, ablate=(),
           sbufs=1):
    nc = bacc.Bacc("TRN2", target_bir_lowering=False, debug=False, num_devices=NC)

    ins = dict(
        bh=nc.dram_tensor("bh", [P, lay.nt2p], I16, kind="ExternalInput").ap(),
        wsp=nc.dram_tensor("wsp", [P, EP * lay.wtot], F16, kind="ExternalInput").ap(),
        xt16=nc.dram_tensor("xt16", [64, XTW], F16, kind="ExternalInput").ap(),
        w1t=nc.dram_tensor("w1t", [64, 64], F32, kind="ExternalInput").ap(),
        wd=nc.dram_tensor("wd", [64, 1], F32, kind="ExternalInput").ap(),
        b1=nc.dram_tensor("b1", [64, 1], F32, kind="ExternalInput").ap(),
        bd=nc.dram_tensor("bd", [1, 1], F32, kind="ExternalInput").ap(),
    )
    out_ext = nc.dram_tensor("out", [1, 1], F32, kind="ExternalOutput").ap()
    dbg = None
    if debug_outs:
        dbg = {
            "d_y": nc.dram_tensor("d_y", [P, AOH], F32, kind="ExternalOutput").ap(),
            "d_y2": nc.dram_tensor("d_y2", [P, 8 * AOH], F32, kind="ExternalOutput").ap(),
            "d_prod": nc.dram_tensor("d_prod", [P, EP * 8 * AOH], F32, kind="ExternalOutput").ap(),
            "d_pall": nc.dram_tensor("d_pall", [1, NC * 16], F32, kind="ExternalOutput").ap(),
            "d_logit": nc.dram_tensor("d_logit", [1, 1], F32, kind="ExternalOutput").ap(),
        }

    rg = [list(range(NC))]

    with tile.TileContext(nc) as tc:
        with (
            tc.tile_pool(name="sb", bufs=sbufs) as sb,
            tc.tile_pool(name="g", bufs=5) as gp,
            tc.tile_pool(name="ps", bufs=1, space="PSUM") as ps,
            tc.tile_pool(name="psA", bufs=1, space="PSUM") as psA,
            tc.tile_pool(name="dr", bufs=1, space="DRAM") as dr,
        ):
            acc_s = None
            if acc:
                acc_s = sb.tile([1, 1], F32, tag="accm")
                nc.vector.memset(acc_s[:], 0.0)
            for rep in range(reps):
                _emit_body(
                    nc, sb, gp, ps, psA, dr, rg, ins, lay,
                    out_ext if rep == reps - 1 else None,
                    acc_s=acc_s, dbg=dbg if rep == reps - 1 else None,
                    sj=sj, ablate=ablate,
                )

    nc.compile()
    return nc


def _emit_body(nc, sb, gp, ps, psA, dr, rg, ins, lay, out_ext, acc_s=None, dbg=None,
               sj=4, ablate=()):
    NT2, NB = lay.nt2, lay.nb
    # ---- loads (bh first so the DVE can start early; xt16 gates only y) --
    bh_s = sb.tile([P, lay.nt2p], I16, tag="bh")
    nc.sync.dma_start(out=bh_s[:], in_=ins["bh"])
    wsp_s = sb.tile([P, EP * lay.wtot], F16, tag="wsp")
    nc.sync.dma_start(out=wsp_s[:], in_=ins["wsp"])
    xt_s = sb.tile([64, XTW], F16, tag="xt")
    nc.sync.dma_start(out=xt_s[:], in_=ins["xt16"])
    w1t_s = sb.tile([64, 64], F32, tag="w1t")
    nc.sync.dma_start(out=w1t_s[:], in_=ins["w1t"])
    wd_s = sb.tile([64, 1], F32, tag="wd")
    nc.sync.dma_start(out=wd_s[:], in_=ins["wd"])
    b1_s = sb.tile([64, 1], F32, tag="b1")
    nc.sync.dma_start(out=b1_s[:], in_=ins["b1"])
    bd_s = sb.tile([1, 1], F32, tag="bd")
    nc.sync.dma_start(out=bd_s[:], in_=ins["bd"])

    # ---- head weights: u = W1 @ Wd (fp16 for the y matmuls); c0 = b1.Wd --
    u_ps = ps.tile([64, 1], F32, tag="ups")
    nc.tensor.matmul(out=u_ps[:], lhsT=w1t_s[:], rhs=wd_s[:], start=True, stop=True)
    u16 = sb.tile([64, 1], F16, tag="u16")
    nc.vector.tensor_copy(out=u16[:], in_=u_ps[:])
    c0_ps = ps.tile([1, 1], F32, tag="c0ps")
    nc.tensor.matmul(out=c0_ps[:], lhsT=b1_s[:], rhs=wd_s[:], start=True, stop=True)
    c0_s = sb.tile([1, 1], F32, tag="c0s")
    nc.vector.tensor_copy(out=c0_s[:], in_=c0_ps[:])

    # ---- iota reference for the one-hot (m at [m*TB + t]); int16 so the
    # is_equal runs straight against the int16 b stream (no f16 copies) ---
    io_i = sb.tile([P, GSH * TB], I16, tag="ioi")
    nc.gpsimd.iota(io_i[:], pattern=[[1, GSH], [0, TB]], base=0,
                   channel_multiplier=0)
    zeros16 = sb.tile([P, 8 * AOH], F16, tag="z16")
    nc.vector.memset(zeros16[:], 0.0)

    # ---- psum accumulators ----------------------------------------------
    # Flat hist column c = EP*g + r lives in bank c // (8*AOH); all sj
    # streams share the banks at partition rows 32j.  A start=True "zero
    # matmul" per (bank, stream) with a full-width zeros rhs and M=32
    # initializes data AND has_written over every row/column, so the real
    # tiles accumulate with start=False onto clean zeros and the final
    # product can read whole 128-row tiles.
    y_ps = psA.tile([P, AOH], F32, tag="yps")
    NBANK = EP * 8 * AOH // (8 * AOH)
    hist = [
        psA.tile([P, 8 * AOH], F32, tag=f"h{h}", name=f"hist{h}")
        for h in range(NBANK)
    ]
    # One M=128 start=True zero-matmul per bank: clears has_written and
    # writes zeros over ALL rows and columns in a single instruction, so
    # no concurrent-col-group race can leave stale data under a set bit.
    for h in range(NBANK):
        nc.tensor.matmul(
            out=hist[h][:],
            lhsT=zeros16[:, 0:P],
            rhs=zeros16[:],
            start=True, stop=False,
            tile_position=(0, 0),
            skip_group_check=True,
        )

    # per-tile flat column windows + per-bank segments
    BW = 8 * AOH
    segs = []           # per tile: list of (bank, collo, ncol, rhs_off)
    last_in_bank = {}
    for t in range(NT2):
        c0 = EP * int(lay.g_lo[t])
        ncol = EP * int(lay.span[t])
        o = EP * int(lay.woff[t])
        s = []
        done = 0
        while done < ncol:
            h = (c0 + done) // BW
            lo = (c0 + done) % BW
            n = min(ncol - done, BW - lo)
            s.append((h, lo, n, o + done))
            last_in_bank[h] = (t, len(s) - 1)
            done += n
        segs.append(s)

    # y matmuls in M=32 quarters: quarter m -> PE column group m; emitted
    # in chunks between hist batches (after xt16's DMA has landed).
    y_mms = [(a, m) for a in range(AOH) for m in range(4)]

    def emit_y_chunk(lo, hi):
        if "y" in ablate:
            return
        for a, m in y_mms[lo:hi]:
            nc.tensor.matmul(
                out=y_ps[32 * m:32 * m + 32, a:a + 1],
                lhsT=xt_s[:, a * P + 32 * m: a * P + 32 * m + 32],
                rhs=u16[:],
                start=True, stop=True,
                tile_position=(0, 32 * m),
            )

    # ---- hist: DVE one-hot batches + 4-way col-tiled PE accumulation ----
    nchunk = max(1, NB - 1)
    ychunk = (len(y_mms) + nchunk - 1) // nchunk
    for k in range(NB):
        v01 = gp.tile([P, GSH * TB], F16, tag="v01")
        b_b = bh_s[:, k * TB:(k + 1) * TB].rearrange(
            "p (o t) -> p o t", o=1
        ).to_broadcast([P, GSH, TB])
        nc.vector.tensor_tensor(
            out=v01[:].rearrange("p (m t) -> p m t", t=TB),
            in0=b_b,
            in1=io_i[:].rearrange("p (m t) -> p m t", t=TB),
            op=mybir.AluOpType.is_equal,
        )
        v01_r = v01[:].rearrange("p (m t) -> p m t", t=TB)
        for tl in range(TB):
            t = k * TB + tl
            if t >= NT2:
                break
            j = t % sj
            if "hist" in ablate:
                continue
            for si, (h, lo, n, ro) in enumerate(segs[t]):
                nc.tensor.matmul(
                    out=hist[h][32 * j:32 * j + GSH, lo:lo + n],
                    lhsT=v01_r[:, :, tl],
                    rhs=wsp_s[:, ro:ro + n],
                    start=False,
                    stop=(last_in_bank[h] == (t, si)),
                    tile_position=(0, 32 * j),
                    skip_group_check=True,
                )
        if k >= 1:
            emit_y_chunk((k - 1) * ychunk, k * ychunk)

    # ---- y remap: y128[b7, a] -> y2[32j + (g&15... b], g] per col group -
    # n = a*128 + b7 = g*16 + b with g = a*8 + (b7>>4), b = b7 & 15.
    y2_s = sb.tile([P, 8 * AOH], F16, tag="y2")
    nc.vector.memset(y2_s[:], 0.0)
    if "y" not in ablate:
        y_sb = sb.tile([P, AOH], F32, tag="ysb")
        nc.vector.tensor_copy(out=y_sb[:], in_=y_ps[:])
        if dbg is not None:
            nc.sync.dma_start(out=dbg["d_y"], in_=y_sb[:])
        y_dr = dr.tile([8, GSH, AOH], F32, tag="ydr")
        nc.sync.dma_start(out=y_dr[:].rearrange("q b a -> (q b) a"), in_=y_sb[:])
        for j in range(4):
            # SWDGE dma casts f32 -> f16 in flight
            nc.gpsimd.dma_start(
                out=y2_s[32 * j:32 * j + GSH, :],
                in_=y_dr[:].rearrange("q b a -> b q a"),
            )
        if dbg is not None:
            d_y2f = sb.tile([P, 8 * AOH], F32, tag="dy2f")
            nc.vector.tensor_copy(out=d_y2f[:], in_=y2_s[:])
            nc.sync.dma_start(out=dbg["d_y2"], in_=d_y2f[:])

    # ---- logit partial = sum_j sum(hist_j * y2_j) -----------------------
    prodall = sb.tile([P, EP * 8 * AOH], F16, tag="prod")
    if "hist" in ablate:
        nc.vector.memset(prodall[:], 0.0)
    else:
        # ScalarE evacuates each bank as fp16 (it's otherwise idle); the
        # DVE multiply broadcasts y2 over the EP sub-columns.
        hc = sb.tile([P, EP * 8 * AOH], F16, tag="hc")
        for h in range(NBANK):
            nc.scalar.activation(
                out=hc[:, h * BW:(h + 1) * BW],
                in_=hist[h][:],
                func=mybir.ActivationFunctionType.Copy,
            )
        nc.vector.tensor_tensor(
            out=prodall[:].rearrange("p (g r) -> p g r", r=EP),
            in0=hc[:].rearrange("p (g r) -> p g r", r=EP),
            in1=y2_s[:].rearrange("p (g o) -> p g o", o=1).to_broadcast(
                [P, 8 * AOH, EP]
            ),
            op=mybir.AluOpType.mult,
        )
    if dbg is not None:
        nc.gpsimd.dma_start(out=dbg["d_prod"], in_=prodall[:])
    pr = sb.tile([P, 1], F32, tag="pr")
    nc.vector.tensor_reduce(
        out=pr[:], in_=prodall[:], axis=mybir.AxisListType.X,
        op=mybir.AluOpType.add,
    )
    ones_s = sb.tile([P, 1], F32, tag="ones")
    nc.vector.memset(ones_s[:], 1.0)
    tot_ps = ps.tile([1, 1], F32, tag="totps")
    nc.tensor.matmul(out=tot_ps[:], lhsT=pr[:], rhs=ones_s[:], start=True, stop=True)
    part = sb.tile([1, 1], F32, tag="part")
    nc.vector.tensor_copy(out=part[:], in_=tot_ps[:])

    _emit_tail(nc, sb, dr, rg, ins, out_ext, part, c0_s, bd_s, acc_s, dbg,
               ablate=ablate)


def _emit_tail(nc, sb, dr, rg, ins, out_ext, part, c0_s, bd_s, acc_s, dbg,
               ablate=()):
    # ---- AllGather the 8 scalar partials --------------------------------
    if "coll" in ablate:
        tot_s = part
        pall_s = None
    else:
        pc_s = sb.tile([1, 16], F32, tag="pc")
        nc.vector.memset(pc_s[:], 0.0)
        nc.vector.tensor_copy(out=pc_s[:, 0:1], in_=part[:])
        p_dr = dr.tile([1, 16], F32, tag="pdr")
        nc.sync.dma_start(out=p_dr[:], in_=pc_s[:])
        pall_dr = dr.tile([1, NC * 16], F32, tag="palldr")
        nc.gpsimd.collective_compute(
            "AllGather",
            mybir.AluOpType.bypass,
            replica_groups=rg,
            ins=[p_dr.opt()],
            outs=[pall_dr.opt()],
        )
        pall_s = sb.tile([1, NC * 16], F32, tag="palls")
        nc.sync.dma_start(out=pall_s[:], in_=pall_dr[:])

        tot_s = sb.tile([1, 1], F32, tag="tot")
        nc.vector.tensor_reduce(
            out=tot_s[:], in_=pall_s[:], axis=mybir.AxisListType.X,
            op=mybir.AluOpType.add,
        )
    c1_s = sb.tile([1, 1], F32, tag="c1")
    nc.vector.tensor_scalar(
        out=c1_s[:], in0=c0_s[:], scalar1=float(N_NODES), scalar2=None,
        op0=mybir.AluOpType.mult,
    )
    logit_s = sb.tile([1, 1], F32, tag="logit")
    nc.vector.tensor_tensor(
        out=logit_s[:], in0=tot_s[:], in1=c1_s[:], op=mybir.AluOpType.add
    )
    nc.vector.tensor_tensor(
        out=logit_s[:], in0=logit_s[:], in1=bd_s[:], op=mybir.AluOpType.add
    )
    if dbg is not None:
        if pall_s is not None:
            nc.sync.dma_start(out=dbg["d_pall"], in_=pall_s[:])
        nc.sync.dma_start(out=dbg["d_logit"], in_=logit_s[:])

    if acc_s is not None:
        nc.vector.tensor_tensor(
            out=acc_s[:], in0=acc_s[:], in1=logit_s[:], op=mybir.AluOpType.add
        )
        if out_ext is not None:
            nc.sync.dma_start(out=out_ext, in_=acc_s[:])
        return
    out_s = sb.tile([1, 1], F32, tag="outs")
    nc.scalar.activation(
        out=out_s[:], in_=logit_s[:], func=mybir.ActivationFunctionType.Sigmoid
    )
    if out_ext is not None:
        nc.sync.dma_start(out=out_ext, in_=out_s[:])


def _prepare(x, edge_weight, W1, b1, Wd, bd, src):
    x = np.ascontiguousarray(x, dtype=np.float32)
    edge_weight = np.ascontiguousarray(edge_weight, dtype=np.float32)
    src = np.ascontiguousarray(src, dtype=np.int64)
    w1t = np.ascontiguousarray(np.asarray(W1, dtype=np.float32).T)
    wdr = np.ascontiguousarray(Wd, dtype=np.float32).reshape(64, 1)
    b1r = np.ascontiguousarray(b1, dtype=np.float32).reshape(64, 1)
    bdr = np.ascontiguousarray(bd, dtype=np.float32).reshape(1, 1)

    core = src // NSH
    per_core = []
    pcnts = np.zeros((NC, NG), np.int64)
    for c in range(NC):
        sel = core == c
        l = (src[sel] - c * NSH).astype(np.int64)
        w = edge_weight[sel]
        g = ((l >> 4) & 7) * AOH + (l >> 7)
        b = l & 15
        order = np.lexsort((b, g))
        g, b, w, l = g[order], b[order], w[order], l[order]
        # node runs (same l) are adjacent after the (g, b) sort; pack EP
        # edges of one node per slot (shared one-hot row)
        n = l.size
        newrun = np.empty(n, bool)
        newrun[0] = True
        newrun[1:] = l[1:] != l[:-1]
        run_id = np.cumsum(newrun) - 1
        run_start = np.flatnonzero(newrun)
        rr = np.arange(n) - run_start[run_id]
        r = rr % EP
        sl_in_run = rr // EP
        run_cnt = np.diff(np.concatenate([run_start, [n]]))
        run_slots = -(-run_cnt // EP)
        run_g = g[run_start]
        # slot offset of each run within its group (segmented cumsum)
        cum = np.cumsum(run_slots) - run_slots
        run_newg = np.empty(run_g.size, bool)
        run_newg[0] = True
        run_newg[1:] = run_g[1:] != run_g[:-1]
        gfirst = np.maximum.accumulate(
            np.where(run_newg, np.arange(run_g.size), 0)
        )
        run_off_in_g = cum - cum[gfirst]
        slot_in_g = run_off_in_g[run_id] + sl_in_run
        pcnts[c] = np.bincount(run_g, weights=run_slots, minlength=NG
                               ).astype(np.int64)
        per_core.append((g, b, w, r, slot_in_g))

    lay = _Layout(pcnts.max(axis=0))

    maps = []
    for c in range(NC):
        g, b, w, r, slot_in_g = per_core[c]
        slot = lay.goff[g] + slot_in_g
        tt = slot // P
        pp = slot % P
        bh = np.zeros((P, lay.nt2p), np.int16)
        bh[pp, tt] = b
        wsp = np.zeros((P, EP * lay.wtot), np.float16)
        wsp[pp, EP * lay.woff[tt] + EP * (g - lay.g_lo[tt]) + r] = w
        xs = np.zeros((XTW, N_FEAT), np.float32)
        xs[:NSH] = x[c * NSH:(c + 1) * NSH]
        xt16 = np.ascontiguousarray(xs.T.astype(np.float16))
        maps.append({
            "bh": bh, "wsp": wsp, "xt16": xt16,
            "w1t": w1t, "wd": wdr, "b1": b1r, "bd": bdr,
        })
    return lay, maps


def _get_nc(lay, reps=1):
    key = (lay.key, reps)
    if key not in _cache:
        _cache[key] = _build(reps, lay)
    return _cache[key]


def kernel(x, edge_weight, W1, b1, Wd, bd, src, dst, _trace=False, **_ignored):
    lay, maps = _prepare(x, edge_weight, W1, b1, Wd, bd, src)
    nc = _get_nc(lay)
    res = run_bass_kernel_spmd(nc, maps, core_ids=list(range(NC)), trace=_trace)
    out = np.asarray(res.results[0]["out"], dtype=np.float32).reshape(1)
    if _trace:
        return out, res
    return out


if __name__ == "__main__":
    rng = np.random.default_rng(0)
    x = rng.standard_normal((N_NODES, N_FEAT), dtype=np.float32)
    ew = rng.random(N_EDGES, dtype=np.float32)
    W1 = rng.standard_normal((64, 64), dtype=np.float32) / 8.0
    b1 = np.zeros(64, np.float32)
    Wd = rng.standard_normal((64, 1), dtype=np.float32) / 8.0
    bd = np.zeros(1, np.float32)
    src = rng.integers(0, N_NODES, N_EDGES).astype(np.int32)
    dst = rng.integers(0, N_NODES, N_EDGES).astype(np.int32)
    got = kernel(x, ew, W1, b1, Wd, bd, src, dst)
    s = np.bincount(src, weights=ew.astype(np.float64), minlength=N_NODES)
    y = x.astype(np.float64) @ (W1.astype(np.float64) @ Wd.astype(np.float64))
    logit = s @ y.reshape(-1) + N_NODES * float(b1 @ Wd) + float(bd[0])
    want = 1.0 / (1.0 + np.exp(-logit))
    print("got", got, "want", want)
